# revision 14
# baseline (speedup 1.0000x reference)
"""Trainium2 Bass kernel for nn_DGMMLoss (retrieval_knn).

Reference computation (see problem statement):
  1. x_ul = lam*x + (1-lam)*x[perm]; pseudo-label via mode of 11-NN labels
  2. concat; per-class means; gaussian-mixture loss term
  3. kNN regularizer: mode of 3-NN (self-excluded) labels, MSE
  loss = loss_gm + 0.01 * loss_knn

Device strategy (8 NeuronCores, data-parallel over query rows):
  - Scores s[q,r] = 2*q.r - ||r||^2 computed with bf16 matmuls (fp32 psum).
  - Per-row k-th largest via DVE max8 (+match_replace for k=11) -> threshold t.
  - mask[q,r] = s >= t, built in transposed orientation (PE transpose of
    score tiles) so per-class counts = maskT.T @ onehot(y) runs on the PE.
  - mode = argmax_c counts (first max = smallest class on ties, matching
    torch.mode semantics), via reduce_max / select / reduce_min on DVE.
  - GM branch: pi = exp(q.mu - aa/2)*exp(-||mu||^2/2), row-normalized;
    per-row sum((pi - onehot)^2) computed on device.
Host does only O(N*D) glue: x_ul, norms, onehot packing, per-class means,
final scalar assembly.
"""

import math
from contextlib import ExitStack

import numpy as np
import ml_dtypes

import concourse.bass as bass
import concourse.bacc as bacc
import concourse.tile as tile
import concourse.mybir as mybir
from concourse.bass_utils import run_bass_kernel_spmd
from concourse.masks import make_identity

P = 128
NCORES = 8
CLASSES = 100
F32 = mybir.dt.float32
BF16 = mybir.dt.bfloat16
BF16_NP = ml_dtypes.bfloat16
ALU = mybir.AluOpType
AX = mybir.AxisListType


def build_program(R, Q, D, C, k, self_exclude, gm, n_cores=NCORES):
    """One phase of the pipeline as a Bass/Tile program (SPMD over cores).

    R: number of reference rows (shared across cores)
    Q: number of query rows handled by this core
    k: keep the k nearest (largest score) refs per query row
    self_exclude: subtract the query's own label from the counts (knn branch)
    gm: also compute the per-row gaussian-mixture loss term
    """
    DCH, RT, RCH, QB = D // P, R // P, R // 512, Q // P
    assert D % P == 0 and R % 512 == 0 and Q % P == 0 and k <= 16

    nc = bacc.Bacc(
        "TRN2", target_bir_lowering=False, debug=False, num_devices=n_cores
    )
    xT_ap = nc.dram_tensor("xT", [P, DCH * R], BF16, kind="ExternalInput").ap()
    qT_ap = nc.dram_tensor("qT", [P, DCH * Q], BF16, kind="ExternalInput").ap()
    bb_ap = nc.dram_tensor("bbr", [P, R], F32, kind="ExternalInput").ap()
    yoh_ap = nc.dram_tensor("yoh", [P, RT * C], BF16, kind="ExternalInput").ap()
    io_ap = nc.dram_tensor("iotaf", [P, C], F32, kind="ExternalInput").ap()
    nqaux = (2 * QB) if gm else QB
    qaux_ap = (
        nc.dram_tensor("qaux", [P, nqaux], F32, kind="ExternalInput").ap()
        if (self_exclude or gm)
        else None
    )
    muT_ap = emu_ap = None
    if gm:
        muT_ap = nc.dram_tensor("muT", [P, DCH * C], BF16, kind="ExternalInput").ap()
        emu_ap = nc.dram_tensor("emu", [P, C], F32, kind="ExternalInput").ap()
    ym_ap = nc.dram_tensor("ymode", [QB, P, 1], F32, kind="ExternalOutput").ap()
    lg_ap = (
        nc.dram_tensor("lgm", [QB, P, 1], F32, kind="ExternalOutput").ap()
        if gm
        else None
    )

    with tile.TileContext(nc) as tc, ExitStack() as ctx:
        consts = ctx.enter_context(tc.tile_pool(name="consts", bufs=1))
        sbig = ctx.enter_context(tc.tile_pool(name="sbig", bufs=2))
        maskp = ctx.enter_context(tc.tile_pool(name="maskp", bufs=1))
        small = ctx.enter_context(tc.tile_pool(name="small", bufs=1))
        psS_p = ctx.enter_context(tc.tile_pool(name="psS", bufs=2, space="PSUM"))
        psT_p = ctx.enter_context(tc.tile_pool(name="psT", bufs=2, space="PSUM"))
        psC_p = ctx.enter_context(tc.tile_pool(name="psC", bufs=1, space="PSUM"))
        psM_p = ctx.enter_context(tc.tile_pool(name="psM", bufs=1, space="PSUM"))
        psG_p = (
            ctx.enter_context(tc.tile_pool(name="psG", bufs=1, space="PSUM"))
            if gm
            else None
        )

        ident = consts.tile([P, P], F32, name="ident", tag="ident")
        make_identity(nc, ident)

        # Tiny "touch" ops absorb DMA-queue waits into dedicated copies so the
        # wide compute instructions (1-2 HW wait slots) only wait on engine
        # semaphores.
        tchV = consts.tile([1, 1], F32, name="tchV", tag="tchV")
        tchA = consts.tile([1, 1], F32, name="tchA", tag="tchA")

        def dve_touch(ap):
            nc.vector.tensor_copy(tchV[:], ap[0:1, 0:1])

        def act_touch(ap):
            nc.scalar.copy(tchA[:], ap[0:1, 0:1])

        # PE touch of the identity so later transposes don't carry its wait.
        psI = psM_p.tile([1, P], F32, name="psI", tag="psMI")
        nc.tensor.transpose(psI[:], ident[:, 0:1], ident[:])

        # DMA constants in. One dma_start per tile (Tile deps are per-tile, and
        # matmuls only have ~2 wait slots); big constants are split into
        # separate ref-group tiles so compute can start after the first group.
        GROUP = min(R, 2048)
        NG = R // GROUP
        xTs = [[None] * NG for _ in range(DCH)]
        for g in range(NG):
            for d in range(DCH):
                t = consts.tile(
                    [P, GROUP], BF16, name=f"xTs{d}_{g}", tag=f"xTs{d}_{g}"
                )
                nc.sync.dma_start(
                    t[:], xT_ap[:, d * R + g * GROUP: d * R + (g + 1) * GROUP]
                )
                xTs[d][g] = t
        qTt = consts.tile([P, DCH * Q], BF16, name="qTt", tag="qTt")
        nc.sync.dma_start(qTt[:], qT_ap[:])
        bbts = []
        for g in range(NG):
            t = consts.tile([P, GROUP], F32, name=f"bbt{g}", tag=f"bbt{g}")
            nc.sync.dma_start(t[:], bb_ap[:, g * GROUP:(g + 1) * GROUP])
            bbts.append(t)
        yoht = consts.tile([P, RT * C], BF16, name="yoht", tag="yoht")
        nc.sync.dma_start(yoht[:], yoh_ap[:])
        iot = consts.tile([P, C], F32, name="iot", tag="iot")
        nc.sync.dma_start(iot[:], io_ap[:])
        if qaux_ap is not None:
            qauxt = consts.tile([P, nqaux], F32, name="qauxt", tag="qauxt")
            nc.sync.dma_start(qauxt[:], qaux_ap[:])
        if gm:
            muTt = consts.tile([P, DCH * C], BF16, name="muTt", tag="muTt")
            nc.sync.dma_start(muTt[:], muT_ap[:])
            emut = consts.tile([P, C], F32, name="emut", tag="emut")
            nc.sync.dma_start(emut[:], emu_ap[:])
        for g in range(NG):
            dve_touch(bbts[g])
        dve_touch(iot)
        if qaux_ap is not None:
            dve_touch(qauxt)
            act_touch(qauxt)
        if gm:
            dve_touch(emut)

        for b in range(QB):
            # ---- scores S[q, r] = 2*q.r - bb_r for this 128-query block ----
            S = sbig.tile([P, R], F32, name="S", tag="S")
            for j in range(RCH):
                g, go = (j * 512) // GROUP, (j * 512) % GROUP
                ps = psS_p.tile([P, 512], F32, name="psS", tag="psS")
                for d in range(DCH):
                    nc.tensor.matmul(
                        ps[:],
                        qTt[:, d * Q + b * P: d * Q + (b + 1) * P],
                        xTs[d][g][:, go:go + 512],
                        start=(d == 0),
                        stop=(d == DCH - 1),
                    )
                nc.vector.scalar_tensor_tensor(
                    out=S[:, j * 512:(j + 1) * 512],
                    in0=ps[:],
                    scalar=2.0,
                    in1=bbts[g][:, go:go + 512],
                    op0=ALU.mult,
                    op1=ALU.subtract,
                )
            # ---- threshold t = k-th largest score of the row ----
            m1 = small.tile([P, 8], F32, name="m1", tag="m1", bufs=2)
            nc.vector.max(out=m1[:], in_=S[:])
            if k <= 8:
                mt, col = m1, k - 1
            else:
                Ssc = sbig.tile([P, R], F32, name="Ssc", tag="Ssc", bufs=1)
                nc.vector.match_replace(
                    out=Ssc[:], in_to_replace=m1[:], in_values=S[:], imm_value=-1e30
                )
                m2 = small.tile([P, 8], F32, name="m2", tag="m2", bufs=2)
                nc.vector.max(out=m2[:], in_=Ssc[:])
                mt, col = m2, k - 9
            # t as a broadcast row: transpose the picked column -> [1, 128]
            psm = psM_p.tile([1, P], F32, name="psM", tag="psM")
            nc.tensor.transpose(psm[:], mt[:, col:col + 1], ident[:])
            trow = small.tile([1, P], F32, name="trow", tag="trow", bufs=2)
            nc.scalar.copy(trow[:], psm[:])
            tb = small.tile([P, P], F32, name="tb", tag="tb", bufs=2)
            nc.gpsimd.partition_broadcast(tb[:], trow[:])
            dve_touch(tb)
            # ---- transposed mask maskT[r, q] = S[q, r] >= t_q ----
            maskT = maskp.tile([P, RT * P], BF16, name="maskT", tag="maskT")
            for i in range(RT):
                pst = psT_p.tile([P, P], F32, name="psT", tag="psT")
                nc.tensor.transpose(pst[:], S[:, i * P:(i + 1) * P], ident[:])
                nc.vector.tensor_tensor(
                    out=maskT[:, i * P:(i + 1) * P],
                    in0=pst[:],
                    in1=tb[:],
                    op=ALU.is_ge,
                )
            # ---- per-class counts = maskT.T @ onehot(y) ----
            psc = psC_p.tile([P, C], F32, name="psC", tag="psC")
            for i in range(RT):
                nc.tensor.matmul(
                    psc[:],
                    maskT[:, i * P:(i + 1) * P],
                    yoht[:, i * C:(i + 1) * C],
                    start=(i == 0),
                    stop=(i == RT - 1),
                )
            counts = small.tile([P, C], F32, name="counts", tag="counts")
            if self_exclude or gm:
                yh = small.tile([P, C], F32, name="yh", tag="yh")
                nc.vector.tensor_scalar(
                    out=yh[:],
                    in0=iot[:],
                    scalar1=qauxt[:, b:b + 1],
                    scalar2=None,
                    op0=ALU.is_equal,
                )
            if self_exclude:
                nc.vector.tensor_sub(counts[:], psc[:], yh[:])
            else:
                nc.vector.tensor_copy(counts[:], psc[:])
            # ---- mode = first argmax of counts ----
            maxc = small.tile([P, 1], F32, name="maxc", tag="maxc")
            nc.vector.reduce_max(maxc[:], counts[:], axis=AX.X)
            lt01 = small.tile([P, C], F32, name="lt01", tag="lt01")
            nc.vector.tensor_scalar(
                out=lt01[:], in0=counts[:], scalar1=maxc[:], scalar2=None,
                op0=ALU.is_lt,
            )
            cand = small.tile([P, C], F32, name="cand", tag="cand")
            nc.vector.scalar_tensor_tensor(
                out=cand[:], in0=lt01[:], scalar=1e9, in1=iot[:],
                op0=ALU.mult, op1=ALU.add,
            )
            ym = small.tile([P, 1], F32, name="ym", tag="ym")
            nc.vector.tensor_reduce(ym[:], cand[:], axis=AX.X, op=ALU.min)
            nc.sync.dma_start(ym_ap[b], ym[:])
            # ---- gaussian-mixture per-row loss ----
            if gm:
                psg = psG_p.tile([P, C], F32, name="psG", tag="psG")
                for d in range(DCH):
                    nc.tensor.matmul(
                        psg[:],
                        qTt[:, d * Q + b * P: d * Q + (b + 1) * P],
                        muTt[:, d * C:(d + 1) * C],
                        start=(d == 0),
                        stop=(d == DCH - 1),
                    )
                eg = small.tile([P, C], F32, name="eg", tag="eg")
                nc.scalar.activation(
                    eg[:], psg[:], mybir.ActivationFunctionType.Exp,
                    bias=qauxt[:, QB + b:QB + b + 1], scale=1.0,
                )
                piu = small.tile([P, C], F32, name="piu", tag="piu")
                nc.vector.tensor_mul(piu[:], eg[:], emut[:])
                srow = small.tile([P, 1], F32, name="srow", tag="srow")
                nc.vector.reduce_sum(srow[:], piu[:], axis=AX.X)
                nc.vector.tensor_scalar_add(srow[:], srow[:], 1e-15)
                rec = small.tile([P, 1], F32, name="rec", tag="rec")
                nc.vector.reciprocal(rec[:], srow[:])
                pin = small.tile([P, C], F32, name="pin", tag="pin")
                nc.vector.tensor_scalar(
                    out=pin[:], in0=piu[:], scalar1=rec[:], scalar2=None,
                    op0=ALU.mult,
                )
                diff = small.tile([P, C], F32, name="diff", tag="diff")
                nc.vector.tensor_sub(diff[:], pin[:], yh[:])
                sq = small.tile([P, C], F32, name="sq", tag="sq")
                nc.vector.tensor_mul(sq[:], diff[:], diff[:])
                lg = small.tile([P, 1], F32, name="lg", tag="lg")
                nc.vector.reduce_sum(lg[:], sq[:], axis=AX.X)
                nc.sync.dma_start(lg_ap[b], lg[:])
    nc.compile()
    return nc


# ---------------- host-side packing helpers ----------------

def pack_T(m):
    """[R, D] fp32 -> bf16 [P, (D//P)*R]: column block d holds rows d*P..(d+1)*P
    of m.T (i.e. element (p, d*R + r) = m[r, d*P + p])."""
    R, D = m.shape
    DCH = D // P
    mt = np.ascontiguousarray(m.T.astype(BF16_NP))  # [D, R]
    return np.ascontiguousarray(
        mt.reshape(DCH, P, R).transpose(1, 0, 2).reshape(P, DCH * R)
    )


def pack_yoh(yi, C):
    Rn = yi.shape[0]
    RT = Rn // P
    yoh = np.zeros((Rn, C), dtype=BF16_NP)
    yoh[np.arange(Rn), yi] = 1
    return np.ascontiguousarray(
        yoh.reshape(RT, P, C).transpose(1, 0, 2).reshape(P, RT * C)
    )


def pack_cols(v):
    """[Q] -> [P, Q//P] fp32: column b = v[b*P:(b+1)*P]."""
    QB = v.shape[0] // P
    return np.ascontiguousarray(v.reshape(QB, P).T.astype(np.float32))


_PROGRAMS = {}


def _get_program(key, builder):
    if key not in _PROGRAMS:
        _PROGRAMS[key] = builder()
    return _PROGRAMS[key]


def kernel(x, y, lam, perm):
    x = np.asarray(x, dtype=np.float32)
    y = np.asarray(y, dtype=np.float32)
    lam = np.float32(np.asarray(lam))
    perm = np.asarray(perm, dtype=np.int32)
    N, D = x.shape
    C = CLASSES
    x_ul = (x * lam + x[perm] * (np.float32(1.0) - lam)).astype(np.float32)

    iota_in = np.ascontiguousarray(
        np.broadcast_to(np.arange(C, dtype=np.float32), (P, C))
    )

    # ---------------- phase 1: pseudo-labels via 11-NN mode ----------------
    QA = N // NCORES
    ncA = _get_program(
        ("A", N, QA, D), lambda: build_program(N, QA, D, C, 11, False, False)
    )
    xT_in = pack_T(x)
    bb_x = (x.astype(np.float64) ** 2).sum(1).astype(np.float32)
    bb_in = np.ascontiguousarray(np.broadcast_to(bb_x, (P, N)))
    yoh_in = pack_yoh(y.astype(np.int32), C)
    in_maps = []
    for c in range(NCORES):
        in_maps.append(
            {
                "xT": xT_in,
                "qT": pack_T(x_ul[c * QA:(c + 1) * QA]),
                "bbr": bb_in,
                "yoh": yoh_in,
                "iotaf": iota_in,
            }
        )
    resA = run_bass_kernel_spmd(ncA, in_maps, core_ids=list(range(NCORES)))
    y_ul = np.concatenate(
        [r["ymode"].reshape(QA) for r in resA.results]
    ).astype(np.float32)

    # ---------------- host glue: per-class means ----------------
    xc = np.concatenate([x, x_ul], axis=0)
    yc = np.concatenate([y, y_ul], axis=0)
    num = xc.shape[0]
    yi = yc.astype(np.int32)
    counts = np.bincount(yi, minlength=C).astype(np.float32)
    mu = np.zeros((C, D), dtype=np.float32)
    np.add.at(mu, yi, xc)
    mu = mu / np.maximum(counts, 1.0)[:, None]
    bbm = (mu.astype(np.float64) ** 2).sum(1)
    emu = (np.exp(-bbm / 2.0) * (counts > 0)).astype(np.float32)
    emu_in = np.ascontiguousarray(np.broadcast_to(emu, (P, C)))
    aa = (xc.astype(np.float64) ** 2).sum(1).astype(np.float32)
    bb_in2 = np.ascontiguousarray(np.broadcast_to(aa, (P, num)))
    yoh2_in = pack_yoh(yi, C)
    muT_in = pack_T(mu)
    xcT_in = pack_T(xc)

    # ---------------- phase 2: 3-NN mode + gm loss rows ----------------
    QB_ = num // NCORES
    ncB = _get_program(
        ("B", num, QB_, D), lambda: build_program(num, QB_, D, C, 4, True, True)
    )
    in_maps = []
    for c in range(NCORES):
        sl = slice(c * QB_, (c + 1) * QB_)
        qaux = np.concatenate(
            [pack_cols(yc[sl]), pack_cols(-0.5 * aa[sl])], axis=1
        ).astype(np.float32)
        in_maps.append(
            {
                "xT": xcT_in,
                "qT": pack_T(xc[sl]),
                "bbr": bb_in2,
                "yoh": yoh2_in,
                "iotaf": iota_in,
                "qaux": np.ascontiguousarray(qaux),
                "muT": muT_in,
                "emu": emu_in,
            }
        )
    resB = run_bass_kernel_spmd(ncB, in_maps, core_ids=list(range(NCORES)))
    y_ng = np.concatenate(
        [r["ymode"].reshape(QB_) for r in resB.results]
    ).astype(np.float32)
    lgm_rows = np.concatenate([r["lgm"].reshape(QB_) for r in resB.results])

    loss_gm = np.float32(lgm_rows.mean(dtype=np.float64))
    loss_knn = np.float32(((y_ng - yc) ** 2).mean(dtype=np.float64))
    return np.float32(loss_gm + np.float32(0.01) * loss_knn)


# revision 32
# speedup vs baseline: 1.4210x; 1.4210x over previous
"""Trainium2 Bass kernel for nn_DGMMLoss (retrieval_knn).

Reference computation (see problem statement):
  1. x_ul = lam*x + (1-lam)*x[perm]; pseudo-label via mode of 11-NN labels
  2. concat; per-class means; gaussian-mixture loss term
  3. kNN regularizer: mode of 3-NN (self-excluded) labels, MSE
  loss = loss_gm + 0.01 * loss_knn

Device strategy (8 NeuronCores, data-parallel over query rows):
  - Scores s[q,r] = 2*q.r - ||r||^2 computed with bf16 matmuls (fp32 psum).
  - Per-row k-th largest via DVE max8 (+match_replace for k=11) -> threshold t.
  - mask[q,r] = s >= t, built in transposed orientation (PE transpose of
    score tiles) so per-class counts = maskT.T @ onehot(y) runs on the PE.
  - mode = argmax_c counts (first max = smallest class on ties, matching
    torch.mode semantics), via reduce_max / select / reduce_min on DVE.
  - GM branch: pi = exp(q.mu - aa/2)*exp(-||mu||^2/2), row-normalized;
    per-row sum((pi - onehot)^2) computed on device.
Host does only O(N*D) glue: x_ul, norms, onehot packing, per-class means,
final scalar assembly.
"""

import math
from contextlib import ExitStack

import numpy as np
import ml_dtypes

import concourse.bass as bass
import concourse.bacc as bacc
import concourse.tile as tile
import concourse.mybir as mybir
from concourse.bass_utils import run_bass_kernel_spmd
from concourse.masks import make_identity

P = 128
NCORES = 8
CLASSES = 100
F32 = mybir.dt.float32
BF16 = mybir.dt.bfloat16
BF16_NP = ml_dtypes.bfloat16
ALU = mybir.AluOpType
AX = mybir.AxisListType


def build_program(R, Q, D, C, k, self_exclude, gm, n_cores=NCORES, _stages=3):
    """One phase of the pipeline as a Bass/Tile program (SPMD over cores).

    R: number of reference rows (shared across cores)
    Q: number of query rows handled by this core
    k: keep the k nearest (largest score) refs per query row
    self_exclude: subtract the query's own label from the counts (knn branch)
    gm: also compute the per-row gaussian-mixture loss term
    """
    DCH, RT, RCH, QB = D // P, R // P, R // 512, Q // P
    assert D % P == 0 and R % 512 == 0 and Q % P == 0 and k <= 16

    nc = bacc.Bacc(
        "TRN2", target_bir_lowering=False, debug=False, num_devices=n_cores
    )
    xT_ap = nc.dram_tensor("xT", [P, DCH * R], BF16, kind="ExternalInput").ap()
    qT_ap = nc.dram_tensor("qT", [P, DCH * Q], BF16, kind="ExternalInput").ap()
    # -||r||^2/2 split into bf16 hi+lo rows, folded into the score matmul as
    # an augmented K=2 contraction against a column of ones.
    bb_ap = nc.dram_tensor("bbhl", [2, R], BF16, kind="ExternalInput").ap()
    yoh_ap = nc.dram_tensor("yoh", [P, RT * C], BF16, kind="ExternalInput").ap()
    io_ap = nc.dram_tensor("iotaf", [P, C], F32, kind="ExternalInput").ap()
    nqaux = (2 * QB) if gm else QB
    qaux_ap = (
        nc.dram_tensor("qaux", [P, nqaux], F32, kind="ExternalInput").ap()
        if (self_exclude or gm)
        else None
    )
    muT_ap = emu_ap = None
    if gm:
        muT_ap = nc.dram_tensor("muT", [P, DCH * C], BF16, kind="ExternalInput").ap()
        emu_ap = nc.dram_tensor("emu", [P, C], F32, kind="ExternalInput").ap()
    ym_ap = nc.dram_tensor("ymode", [QB, P, 1], F32, kind="ExternalOutput").ap()
    lg_ap = (
        nc.dram_tensor("lgm", [QB, P, 1], F32, kind="ExternalOutput").ap()
        if gm
        else None
    )

    with tile.TileContext(nc) as tc, ExitStack() as ctx:
        consts = ctx.enter_context(tc.tile_pool(name="consts", bufs=1))
        sbig = ctx.enter_context(tc.tile_pool(name="sbig", bufs=2))
        maskp = ctx.enter_context(tc.tile_pool(name="maskp", bufs=1))
        small = ctx.enter_context(tc.tile_pool(name="small", bufs=1))
        psS_p = ctx.enter_context(tc.tile_pool(name="psS", bufs=2, space="PSUM"))
        psT_p = ctx.enter_context(tc.tile_pool(name="psT", bufs=3, space="PSUM"))
        psC_p = ctx.enter_context(tc.tile_pool(name="psC", bufs=1, space="PSUM"))
        psG_p = (
            ctx.enter_context(tc.tile_pool(name="psG", bufs=1, space="PSUM"))
            if gm
            else None
        )

        identb = consts.tile([P, P], BF16, name="identb", tag="identb")
        make_identity(nc, identb)

        # Tiny "touch" ops absorb DMA-queue waits into dedicated copies so the
        # wide compute instructions (1-2 HW wait slots) only wait on engine
        # semaphores.
        tchV = consts.tile([1, 1], F32, name="tchV", tag="tchV")
        tchA = consts.tile([1, 1], F32, name="tchA", tag="tchA")

        def dve_touch(ap):
            nc.vector.tensor_copy(tchV[:], ap[0:1, 0:1])

        def act_touch(ap):
            nc.scalar.copy(tchA[:], ap[0:1, 0:1])

        # PE touch of the identity so later transposes don't carry its wait.
        psI = psT_p.tile([1, P], BF16, name="psI", tag="psMI", bufs=1)
        nc.tensor.transpose(psI[:], identb[:, 0:1], identb[:])

        # DMA constants in. One dma_start per tile (Tile deps are per-tile, and
        # matmuls only have ~2 wait slots); big constants are split into
        # separate ref-group tiles so compute can start after the first group.
        GROUP = min(R, 2048)
        NG = R // GROUP
        xTs = [[None] * NG for _ in range(DCH)]
        for g in range(NG):
            for d in range(DCH):
                t = consts.tile(
                    [P, GROUP], BF16, name=f"xTs{d}_{g}", tag=f"xTs{d}_{g}"
                )
                nc.sync.dma_start(
                    t[:], xT_ap[:, d * R + g * GROUP: d * R + (g + 1) * GROUP]
                )
                xTs[d][g] = t
        qTt = consts.tile([P, DCH * Q], BF16, name="qTt", tag="qTt")
        nc.sync.dma_start(qTt[:], qT_ap[:])
        ones2 = consts.tile([2, P], BF16, name="ones2", tag="ones2")
        nc.vector.memset(ones2[:], 1.0)
        bbts = []
        for g in range(NG):
            t = consts.tile([2, GROUP], BF16, name=f"bbt{g}", tag=f"bbt{g}")
            nc.sync.dma_start(t[:], bb_ap[:, g * GROUP:(g + 1) * GROUP])
            bbts.append(t)
        yoht = consts.tile([P, RT * C], BF16, name="yoht", tag="yoht")
        nc.sync.dma_start(yoht[:], yoh_ap[:])
        iot = consts.tile([P, C], F32, name="iot", tag="iot")
        nc.sync.dma_start(iot[:], io_ap[:])
        if qaux_ap is not None:
            qauxt = consts.tile([P, nqaux], F32, name="qauxt", tag="qauxt")
            nc.sync.dma_start(qauxt[:], qaux_ap[:])
        if gm:
            muTt = consts.tile([P, DCH * C], BF16, name="muTt", tag="muTt")
            nc.sync.dma_start(muTt[:], muT_ap[:])
            emut = consts.tile([P, C], F32, name="emut", tag="emut")
            nc.sync.dma_start(emut[:], emu_ap[:])
        dve_touch(iot)
        if qaux_ap is not None:
            dve_touch(qauxt)
            act_touch(qauxt)
        if gm:
            dve_touch(emut)

        R2 = R // 2
        HT = RT // 2  # mask tiles per half

        def emit_counts(b, halves):
            """Counts + mode (+ gm) for query block b given its mask halves."""
            psc = psC_p.tile([P, C], F32, name="psC", tag="psC")
            GT = min(8, RT)  # transposes batched per PSUM bank / ACT copy
            for i0 in range(0, RT, GT):
                pst = psT_p.tile([P, GT * P], BF16, name="psT", tag="psT")
                for u in range(GT):
                    i = i0 + u
                    mh = halves[i // HT]
                    lo = (i % HT) * P
                    nc.tensor.transpose(
                        pst[:, u * P:(u + 1) * P], mh[:, lo:lo + P], identb[:]
                    )
                mTg = maskp.tile([P, GT * P], BF16, name="mTg", tag="mTg", bufs=3)
                nc.scalar.copy(mTg[:], pst[:])
                for u in range(GT):
                    i = i0 + u
                    nc.tensor.matmul(
                        psc[:],
                        mTg[:, u * P:(u + 1) * P],
                        yoht[:, i * C:(i + 1) * C],
                        start=(i == 0),
                        stop=(i == RT - 1),
                    )
            counts = small.tile([P, C], F32, name="counts", tag="counts")
            if self_exclude or gm:
                yh = small.tile([P, C], F32, name="yh", tag="yh")
                nc.vector.tensor_scalar(
                    out=yh[:],
                    in0=iot[:],
                    scalar1=qauxt[:, b:b + 1],
                    scalar2=None,
                    op0=ALU.is_equal,
                )
            if self_exclude:
                nc.vector.tensor_sub(counts[:], psc[:], yh[:])
            else:
                nc.vector.tensor_copy(counts[:], psc[:])
            # mode = first argmax of counts
            maxc = small.tile([P, 1], F32, name="maxc", tag="maxc")
            nc.vector.reduce_max(maxc[:], counts[:], axis=AX.X)
            lt01 = small.tile([P, C], F32, name="lt01", tag="lt01")
            nc.vector.tensor_scalar(
                out=lt01[:], in0=counts[:], scalar1=maxc[:], scalar2=None,
                op0=ALU.is_lt,
            )
            cand = small.tile([P, C], F32, name="cand", tag="cand")
            nc.vector.scalar_tensor_tensor(
                out=cand[:], in0=lt01[:], scalar=1e9, in1=iot[:],
                op0=ALU.mult, op1=ALU.add,
            )
            ym = small.tile([P, 1], F32, name="ym", tag="ym")
            nc.vector.tensor_reduce(ym[:], cand[:], axis=AX.X, op=ALU.min)
            nc.sync.dma_start(ym_ap[b], ym[:])
            # gaussian-mixture per-row loss
            if gm:
                psg = psG_p.tile([P, C], F32, name="psG", tag="psG")
                for d in range(DCH):
                    nc.tensor.matmul(
                        psg[:],
                        qTt[:, d * Q + b * P: d * Q + (b + 1) * P],
                        muTt[:, d * C:(d + 1) * C],
                        start=(d == 0),
                        stop=(d == DCH - 1),
                    )
                eg = small.tile([P, C], F32, name="eg", tag="eg")
                nc.scalar.activation(
                    eg[:], psg[:], mybir.ActivationFunctionType.Exp,
                    bias=qauxt[:, QB + b:QB + b + 1], scale=1.0,
                )
                piu = small.tile([P, C], F32, name="piu", tag="piu")
                nc.vector.tensor_mul(piu[:], eg[:], emut[:])
                srow = small.tile([P, 1], F32, name="srow", tag="srow")
                nc.vector.reduce_sum(srow[:], piu[:], axis=AX.X)
                nc.vector.tensor_scalar_add(srow[:], srow[:], 1e-15)
                rec = small.tile([P, 1], F32, name="rec", tag="rec")
                nc.vector.reciprocal(rec[:], srow[:])
                pin = small.tile([P, C], F32, name="pin", tag="pin")
                nc.vector.tensor_scalar(
                    out=pin[:], in0=piu[:], scalar1=rec[:], scalar2=None,
                    op0=ALU.mult,
                )
                diff = small.tile([P, C], F32, name="diff", tag="diff")
                nc.vector.tensor_sub(diff[:], pin[:], yh[:])
                sq = small.tile([P, C], F32, name="sq", tag="sq")
                nc.vector.tensor_mul(sq[:], diff[:], diff[:])
                lg = small.tile([P, 1], F32, name="lg", tag="lg")
                nc.vector.reduce_sum(lg[:], sq[:], axis=AX.X)
                nc.sync.dma_start(lg_ap[b], lg[:])

        # Software pipeline: block b's counts/mode are emitted after block
        # b+1's scores/threshold/compare, so the PE's counts work overlaps the
        # DVE threshold tail of the next block.
        pending = None
        for b in range(QB):
            # ---- scores S[q, r] = q.r - bb_r/2 for this 128-query block
            # (rank-equivalent to 2*q.r - bb_r; bb folded into the matmul) ----
            S = sbig.tile([P, R], F32, name="S", tag="S")
            for j in range(RCH):
                g, go = (j * 512) // GROUP, (j * 512) % GROUP
                ps = psS_p.tile([P, 512], F32, name="psS", tag="psS")
                for d in range(DCH):
                    nc.tensor.matmul(
                        ps[:],
                        qTt[:, d * Q + b * P: d * Q + (b + 1) * P],
                        xTs[d][g][:, go:go + 512],
                        start=(d == 0),
                        stop=False,
                    )
                nc.tensor.matmul(
                    ps[:],
                    ones2[:],
                    bbts[g][:, go:go + 512],
                    start=False,
                    stop=True,
                )
                nc.scalar.copy(S[:, j * 512:(j + 1) * 512], ps[:])
            # ---- threshold t = k-th largest score of the row ----
            if _stages < 2:
                nc.vector.max(out=small.tile([P, 8], F32, name="mdum", tag="mdum"), in_=S[:, 0:512])
                pending = None
                continue
            m1 = small.tile([P, 8], F32, name="m1", tag="m1", bufs=2)
            nc.vector.max(out=m1[:], in_=S[:])
            if k <= 8:
                mt, col = m1, k - 1
            else:
                Ssc = sbig.tile([P, R], F32, name="Ssc", tag="Ssc", bufs=1)
                nc.vector.match_replace(
                    out=Ssc[:], in_to_replace=m1[:], in_values=S[:], imm_value=-1e30
                )
                m2 = small.tile([P, 8], F32, name="m2", tag="m2", bufs=2)
                nc.vector.max(out=m2[:], in_=Ssc[:])
                mt, col = m2, k - 9
            # ---- mask[q, r] = S >= t_q, in two halves for finer overlap ----
            halves = []
            for h in range(2):
                mh = maskp.tile([P, R2], BF16, name="mh", tag="mh", bufs=2)
                nc.vector.tensor_scalar(
                    out=mh[:], in0=S[:, h * R2:(h + 1) * R2],
                    scalar1=mt[:, col:col + 1], scalar2=None, op0=ALU.is_ge,
                )
                halves.append(mh)
            if _stages >= 3 and pending is not None:
                emit_counts(*pending)
            pending = (b, halves)
        if _stages >= 3:
            emit_counts(*pending)
    nc.compile()
    return nc


# ---------------- host-side packing helpers ----------------

def pack_T(m):
    """[R, D] fp32 -> bf16 [P, (D//P)*R]: column block d holds rows d*P..(d+1)*P
    of m.T (i.e. element (p, d*R + r) = m[r, d*P + p])."""
    R, D = m.shape
    DCH = D // P
    mt = np.ascontiguousarray(m.T.astype(BF16_NP))  # [D, R]
    return np.ascontiguousarray(
        mt.reshape(DCH, P, R).transpose(1, 0, 2).reshape(P, DCH * R)
    )


def pack_yoh(yi, C):
    Rn = yi.shape[0]
    RT = Rn // P
    yoh = np.zeros((Rn, C), dtype=BF16_NP)
    yoh[np.arange(Rn), yi] = 1
    return np.ascontiguousarray(
        yoh.reshape(RT, P, C).transpose(1, 0, 2).reshape(P, RT * C)
    )


def pack_bbhl(bb):
    """[R] fp32 -> [2, R] bf16 hi/lo split of -bb/2 (exact to ~2^-17 rel)."""
    t = (-0.5 * bb).astype(np.float32)
    hi = t.astype(BF16_NP)
    lo = (t - hi.astype(np.float32)).astype(BF16_NP)
    return np.ascontiguousarray(np.stack([hi, lo]))


def pack_cols(v):
    """[Q] -> [P, Q//P] fp32: column b = v[b*P:(b+1)*P]."""
    QB = v.shape[0] // P
    return np.ascontiguousarray(v.reshape(QB, P).T.astype(np.float32))


_PROGRAMS = {}
LAST_EXEC_NS = None
_EXEC_NS = {}


def _get_program(key, builder):
    if key not in _PROGRAMS:
        _PROGRAMS[key] = builder()
    return _PROGRAMS[key]


def _run(nc, in_maps, phase):
    import os

    kwargs = {}
    if os.environ.get("KERNEL_TRACE"):
        kwargs = dict(trace=True, trace_cores=[0])
    res = run_bass_kernel_spmd(
        nc, in_maps, core_ids=list(range(NCORES)), **kwargs
    )
    if res.exec_time_ns:
        _EXEC_NS[phase] = res.exec_time_ns
        if res.instructions_and_trace:
            print(f"phase {phase}: {res.exec_time_ns} ns, "
                  f"trace: {res.instructions_and_trace[1]}")
    global LAST_EXEC_NS
    if len(_EXEC_NS) == 2:
        LAST_EXEC_NS = sum(_EXEC_NS.values())
    return res


def kernel(x, y, lam, perm):
    x = np.asarray(x, dtype=np.float32)
    y = np.asarray(y, dtype=np.float32)
    lam = np.float32(np.asarray(lam))
    perm = np.asarray(perm, dtype=np.int32)
    N, D = x.shape
    C = CLASSES
    x_ul = (x * lam + x[perm] * (np.float32(1.0) - lam)).astype(np.float32)

    iota_in = np.ascontiguousarray(
        np.broadcast_to(np.arange(C, dtype=np.float32), (P, C))
    )

    # ---------------- phase 1: pseudo-labels via 11-NN mode ----------------
    QA = N // NCORES
    ncA = _get_program(
        ("A", N, QA, D), lambda: build_program(N, QA, D, C, 11, False, False)
    )
    xT_in = pack_T(x)
    bb_x = (x.astype(np.float64) ** 2).sum(1).astype(np.float32)
    bb_in = pack_bbhl(bb_x)
    yoh_in = pack_yoh(y.astype(np.int32), C)
    in_maps = []
    for c in range(NCORES):
        in_maps.append(
            {
                "xT": xT_in,
                "qT": pack_T(x_ul[c * QA:(c + 1) * QA]),
                "bbhl": bb_in,
                "yoh": yoh_in,
                "iotaf": iota_in,
            }
        )
    resA = _run(ncA, in_maps, "A")
    y_ul = np.concatenate(
        [r["ymode"].reshape(QA) for r in resA.results]
    ).astype(np.float32)

    # ---------------- host glue: per-class means ----------------
    xc = np.concatenate([x, x_ul], axis=0)
    yc = np.concatenate([y, y_ul], axis=0)
    num = xc.shape[0]
    yi = yc.astype(np.int32)
    counts = np.bincount(yi, minlength=C).astype(np.float32)
    mu = np.zeros((C, D), dtype=np.float32)
    np.add.at(mu, yi, xc)
    mu = mu / np.maximum(counts, 1.0)[:, None]
    bbm = (mu.astype(np.float64) ** 2).sum(1)
    emu = (np.exp(-bbm / 2.0) * (counts > 0)).astype(np.float32)
    emu_in = np.ascontiguousarray(np.broadcast_to(emu, (P, C)))
    aa = (xc.astype(np.float64) ** 2).sum(1).astype(np.float32)
    bb_in2 = pack_bbhl(aa)
    yoh2_in = pack_yoh(yi, C)
    muT_in = pack_T(mu)
    xcT_in = pack_T(xc)

    # ---------------- phase 2: 3-NN mode + gm loss rows ----------------
    QB_ = num // NCORES
    ncB = _get_program(
        ("B", num, QB_, D), lambda: build_program(num, QB_, D, C, 4, True, True)
    )
    in_maps = []
    for c in range(NCORES):
        sl = slice(c * QB_, (c + 1) * QB_)
        qaux = np.concatenate(
            [pack_cols(yc[sl]), pack_cols(-0.5 * aa[sl])], axis=1
        ).astype(np.float32)
        in_maps.append(
            {
                "xT": xcT_in,
                "qT": pack_T(xc[sl]),
                "bbhl": bb_in2,
                "yoh": yoh2_in,
                "iotaf": iota_in,
                "qaux": np.ascontiguousarray(qaux),
                "muT": muT_in,
                "emu": emu_in,
            }
        )
    resB = _run(ncB, in_maps, "B")
    y_ng = np.concatenate(
        [r["ymode"].reshape(QB_) for r in resB.results]
    ).astype(np.float32)
    lgm_rows = np.concatenate([r["lgm"].reshape(QB_) for r in resB.results])

    loss_gm = np.float32(lgm_rows.mean(dtype=np.float64))
    loss_knn = np.float32(((y_ng - yc) ** 2).mean(dtype=np.float64))
    return np.float32(loss_gm + np.float32(0.01) * loss_knn)
